# revision 5
# baseline (speedup 1.0000x reference)
"""Trainium2 Bass kernel for nn_AttentionHawkes (B=32, L=2048, D=2048, 8 cores).

Sharding: batch-parallel for context (4 batches per core), output-dim-parallel
for W_in/W_out (256 cols per core). Host pre-transposes weights/query and
precomputes bt=exp(-ab*dt).

Per batch on its core:
  pass A: scores[l] = x[l,:]@q via DVE scalar_tensor_tensor accumulate (f32),
          xb = bf16 copy of x on ACT; softmax via small PE matmuls.
  pass B (per tile, all bf16): rp = relu(c2*xb) on GpSimd tensor_scalar
          (c2 = ae*attn*bt, signed); first KZ d-chunks: z = attn*xb + rp
          fused on DVE then one PE matmul per chunk; remaining chunks:
          two PE matmuls per chunk (attn-weighted xb + ones-weighted rp)
          accumulating into the same PSUM row.
Final: out = tanh([mix|q] @ W_out^T) with the q-half accumulated into a
persistent PSUM bank at startup and the mix-half added in a short tail
after one AllGather.
"""
import sys, os
sys.path.insert(0, "/opt/trn_rl_repo")
import numpy as np

N_CORES = 8
B, L, D = 32, 2048, 2048
BLOC = B // N_CORES          # 4 batches per core
ESL = D // N_CORES           # 256 e-cols of W_in / W_out per core
NLT = L // 128               # 16 l-tiles per batch
NDC = D // 512               # 4 d-chunks of 512

_nc_cache = None


def _build():
    KZ = int(os.environ.get('KZ', '2'))   # d-chunks using the fused-z path
    import concourse.mybir as mybir
    import concourse.tile as tile
    from concourse import bacc
    from concourse.masks import make_identity

    F32 = mybir.dt.float32
    BF16 = mybir.dt.bfloat16
    ALU = mybir.AluOpType
    ACTF = mybir.ActivationFunctionType
    AX = mybir.AxisListType

    nc = bacc.Bacc()

    ctx = nc.dram_tensor("ctx", [BLOC, L, D], F32, kind="ExternalInput")
    qryT = nc.dram_tensor("qryT", [D, B], F32, kind="ExternalInput")
    w_inT = nc.dram_tensor("w_inT", [D, ESL], F32, kind="ExternalInput")
    wmT = nc.dram_tensor("wmT", [D, ESL], BF16, kind="ExternalInput")
    wqT = nc.dram_tensor("wqT", [D, ESL], BF16, kind="ExternalInput")
    btT_in = nc.dram_tensor("btT", [BLOC, 128, NLT], F32, kind="ExternalInput")
    aeb = nc.dram_tensor("aeb", [BLOC, 1], F32, kind="ExternalInput")

    out_sl = nc.dram_tensor("out_sl", [B, ESL], F32, kind="ExternalOutput")
    attn_out = nc.dram_tensor("attn_out", [BLOC, L], F32, kind="ExternalOutput")

    qg_in = nc.dram_tensor("qg_in", [B, ESL], F32)
    qloc = nc.dram_tensor("qloc", [N_CORES, BLOC, ESL], F32)
    qall = nc.dram_tensor("qall", [N_CORES, B, ESL], F32, addr_space="Shared")
    mix_in = nc.dram_tensor("mix_in", [BLOC, D], F32)
    mix_all = nc.dram_tensor("mix_all", [B, D], F32, addr_space="Shared")

    groups = [list(range(N_CORES))]

    with tile.TileContext(nc) as tc:
        with (
            tc.tile_pool(name="cpool", bufs=1) as cpool,
            tc.tile_pool(name="wout", bufs=1) as wout_pool,
            tc.tile_pool(name="pfin", bufs=1, space="PSUM") as pfin_pool,
            tc.tile_pool(name="fin", bufs=1) as fin,
        ):
            ident = cpool.tile([128, 128], F32)
            make_identity(nc, ident[:])
            ones_row = cpool.tile([1, 128], F32)
            nc.vector.memset(ones_row[:], 1.0)
            ones_col = cpool.tile([128, 1], F32)
            nc.vector.memset(ones_col[:], 1.0)
            ones_bf = cpool.tile([128, 2], BF16)
            nc.vector.memset(ones_bf[:], 1.0)

            # resident transposed W_out halves (bf16, 2MB)
            wm = []
            wq = []
            for ct in range(NLT):
                t1 = wout_pool.tile([128, ESL], BF16, tag="wm", name=f"wm{ct}")
                nc.sync.dma_start(t1[:], wmT[ct * 128:(ct + 1) * 128, :])
                wm.append(t1)
            for ct in range(NLT):
                t2 = wout_pool.tile([128, ESL], BF16, tag="wq", name=f"wq{ct}")
                nc.sync.dma_start(t2[:], wqT[ct * 128:(ct + 1) * 128, :])
                wq.append(t2)

            # persistent output-accumulator bank: q-half at startup,
            # mix-half in the tail
            pfin = pfin_pool.tile([B, ESL], F32)

            # ---------- startup: q = query @ W_in[eslice]^T, collectives ----
            with (
                tc.tile_pool(name="wst", bufs=1) as wst,
                tc.tile_pool(name="qst", bufs=1) as qst,
                tc.tile_pool(name="pst", bufs=2, space="PSUM") as pst,
            ):
                wi = []
                for t in range(NLT):
                    wt = wst.tile([128, ESL], F32, tag="wi", name=f"wi{t}")
                    nc.sync.dma_start(wt[:], w_inT[t * 128:(t + 1) * 128, :])
                    wi.append(wt)
                qT_sb = qst.tile([128, NLT * B], F32)
                for t in range(NLT):
                    nc.sync.dma_start(qT_sb[:, t * B:(t + 1) * B],
                                      qryT[t * 128:(t + 1) * 128, :])
                pq = pst.tile([B, ESL], F32, tag="pq")
                for t in range(NLT):
                    nc.tensor.matmul(pq[:], qT_sb[:, t * B:(t + 1) * B],
                                     wi[t][:], start=(t == 0),
                                     stop=(t == NLT - 1))
                q_sb = qst.tile([B, ESL], F32)
                nc.scalar.copy(q_sb[:], pq[:])
                nc.sync.dma_start(qg_in[:], q_sb[:])
                nc.gpsimd.collective_compute(
                    "AllToAll", ALU.bypass, replica_groups=groups,
                    ins=[qg_in.ap().opt()], outs=[qloc.ap().opt()])
                nc.gpsimd.collective_compute(
                    "AllGather", ALU.bypass, replica_groups=groups,
                    ins=[qg_in.ap().opt()], outs=[qall.ap().opt()])

                # qT (all batches, bf16) -> start the pfin chain with q-half
                for ct in range(NLT):
                    i, j = divmod(ct, ESL // 128)
                    qa = qst.tile([B, 128], F32, tag="qa")
                    nc.sync.dma_start(
                        qa[:], qall[i, :, j * 128:(j + 1) * 128])
                    ptq = pst.tile([128, B], F32, tag="ptq")
                    nc.tensor.transpose(ptq[:], qa[:], ident[0:B, 0:B])
                    qtb = qst.tile([128, B], BF16, tag="qtb")
                    nc.scalar.copy(qtb[:], ptq[:])
                    nc.tensor.matmul(pfin[:], qtb[:], wq[ct][:],
                                     start=(ct == 0), stop=False,
                                     skip_group_check=True)

            # ---------- main pools ----------
            with (
                tc.tile_pool(name="xp", bufs=4) as xp,
                tc.tile_pool(name="xb", bufs=NLT) as xbp,
                tc.tile_pool(name="scr", bufs=2) as scr_pool,
                tc.tile_pool(name="rp", bufs=3) as rp_pool,
                tc.tile_pool(name="zp", bufs=3) as zp_pool,
                tc.tile_pool(name="qb", bufs=2) as qb_pool,
                tc.tile_pool(name="small", bufs=2) as small,
                tc.tile_pool(name="pm", bufs=1, space="PSUM") as pm_pool,
                tc.tile_pool(name="ptr", bufs=1, space="PSUM") as ptr_pool,
            ):
                for b in range(BLOC):
                    # q broadcast rows for the stt
                    qb = qb_pool.tile([128, D], F32, tag="qb")
                    for i in range(N_CORES):
                        nc.sync.dma_start(
                            qb[:, i * ESL:(i + 1) * ESL],
                            qloc[i:i + 1, b, :].broadcast_to([128, ESL]))

                    btb = small.tile([128, NLT], F32, tag="btb")
                    nc.sync.dma_start(btb[:], btT_in[b])
                    ae_col = small.tile([128, 1], F32, tag="ae_col")
                    nc.sync.dma_start(ae_col[:],
                                      aeb[b:b + 1, 0:1].broadcast_to([128, 1]))

                    # pass A: load x tiles, scores, bf16 copies
                    scores = small.tile([128, NLT], F32, tag="scores")
                    xbs = []
                    for t in range(NLT):
                        xt = xp.tile([128, D], F32, tag="xt")
                        nc.sync.dma_start(xt[:], ctx[b, t * 128:(t + 1) * 128, :])
                        scr = scr_pool.tile([128, D], BF16, tag="scr")
                        nc.vector.scalar_tensor_tensor(
                            out=scr[:], in0=xt[:], scalar=1.0, in1=qb[:],
                            op0=ALU.mult, op1=ALU.mult,
                            accum_out=scores[:, t:t + 1])
                        xbt = xbp.tile([128, D], BF16, tag="xb")
                        nc.scalar.copy(xbt[:], xt[:])
                        xbs.append(xbt)

                    # softmax over all 2048 scores (PE partition reduces)
                    m1 = small.tile([128, 1], F32, tag="m1")
                    nc.vector.reduce_max(m1[:], scores[:], axis=AX.X)
                    ptm = ptr_pool.tile([1, 128], F32, tag="ptr")
                    nc.tensor.transpose(ptm[:], m1[:], ident[:])
                    mg = small.tile([1, 1], F32, tag="mg")
                    nc.vector.reduce_max(mg[:], ptm[:], axis=AX.X)
                    nc.vector.tensor_scalar_mul(mg[:], mg[:], -1.0)
                    pnb = ptr_pool.tile([128, 1], F32, tag="ptr2")
                    nc.tensor.matmul(pnb[:], ones_row[:], mg[:],
                                     start=True, stop=True)
                    negm = small.tile([128, 1], F32, tag="negm")
                    nc.scalar.copy(negm[:], pnb[:])
                    E = small.tile([128, NLT], F32, tag="E")
                    s1 = small.tile([128, 1], F32, tag="s1")
                    nc.scalar.activation(E[:], scores[:], ACTF.Exp,
                                         bias=negm[:], accum_out=s1[:])
                    pz = ptr_pool.tile([1, 1], F32, tag="ptr")
                    nc.tensor.matmul(pz[:], s1[:, 0:1], ones_col[:, 0:1],
                                     start=True, stop=True)
                    rzg = small.tile([1, 1], F32, tag="rzg")
                    nc.vector.reciprocal(rzg[:], pz[:])
                    prz = ptr_pool.tile([128, 1], F32, tag="ptr2")
                    nc.tensor.matmul(prz[:], ones_row[:], rzg[:],
                                     start=True, stop=True)
                    rz = small.tile([128, 1], F32, tag="rz")
                    nc.scalar.copy(rz[:], prz[:])
                    attn = small.tile([128, NLT], F32, tag="attn")
                    nc.vector.tensor_scalar(out=attn[:], in0=E[:],
                                            scalar1=rz[:], scalar2=None,
                                            op0=ALU.mult)

                    # coefficients: bf16 attn for lhsT, signed c2 = ae*attn*bt
                    attn_bf = small.tile([128, NLT], BF16, tag="attn_bf")
                    nc.vector.tensor_copy(attn_bf[:], attn[:])
                    c2 = small.tile([128, NLT], F32, tag="c2")
                    nc.vector.tensor_tensor(out=c2[:], in0=attn[:],
                                            in1=btb[:], op=ALU.mult)
                    nc.vector.tensor_scalar(out=c2[:], in0=c2[:],
                                            scalar1=ae_col[:], scalar2=None,
                                            op0=ALU.mult)

                    # attn output (transpose to l-major)
                    pat = ptr_pool.tile([NLT, 128], F32, tag="ptr")
                    nc.tensor.transpose(pat[:], attn[:], ident[:])
                    at_sb = small.tile([NLT, 128], F32, tag="at_sb")
                    nc.scalar.copy(at_sb[:], pat[:])
                    nc.sync.dma_start(
                        attn_out[b].rearrange("(t p) -> t p", p=128), at_sb[:])

                    # pass B
                    pms = [pm_pool.tile([2, 512], F32, tag=f"pm{dc}",
                                        name=f"pm{dc}")
                           for dc in range(NDC)]
                    ZW = KZ * 512
                    for t in range(NLT):
                        rpt = rp_pool.tile([128, D], BF16, tag="rp")
                        nc.gpsimd.tensor_scalar(
                            out=rpt[:], in0=xbs[t][:], scalar1=c2[:, t:t + 1],
                            scalar2=0.0, op0=ALU.mult, op1=ALU.max)
                        if KZ > 0:
                            zt = zp_pool.tile([128, ZW], BF16, tag="zt")
                            nc.vector.scalar_tensor_tensor(
                                out=zt[:], in0=xbs[t][:, 0:ZW],
                                scalar=attn[:, t:t + 1], in1=rpt[:, 0:ZW],
                                op0=ALU.mult, op1=ALU.add)
                            for dc in range(KZ):
                                nc.tensor.matmul(
                                    pms[dc][:], ones_bf[:],
                                    zt[:, dc * 512:(dc + 1) * 512],
                                    start=(t == 0), stop=(t == NLT - 1))
                        for dc in range(KZ, NDC):
                            nc.tensor.matmul(
                                pms[dc][:],
                                attn_bf[:, t:t + 1].broadcast_to([128, 2]),
                                xbs[t][:, dc * 512:(dc + 1) * 512],
                                start=(t == 0), stop=False)
                            nc.tensor.matmul(
                                pms[dc][:], ones_bf[:],
                                rpt[:, dc * 512:(dc + 1) * 512],
                                start=False, stop=(t == NLT - 1))

                    # mix row -> DRAM
                    ms = fin.tile([1, D], F32, tag="ms")
                    for dc in range(NDC):
                        nc.scalar.copy(ms[:, dc * 512:(dc + 1) * 512],
                                       pms[dc][0:1, :])
                    nc.sync.dma_start(mix_in[b:b + 1, :], ms[0:1, :])

                # ---------- tail: gather mix, finish pfin, tanh ----------
                nc.gpsimd.collective_compute(
                    "AllGather", ALU.bypass, replica_groups=groups,
                    ins=[mix_in.ap().opt()], outs=[mix_all.ap().opt()])
                comb_sb = fin.tile([B, D], F32, tag="comb")
                nc.sync.dma_start(comb_sb[:], mix_all[:])
                for ct in range(NLT):
                    ptc = ptr_pool.tile([128, B], F32, tag="ptr3")
                    nc.tensor.transpose(
                        ptc[:], comb_sb[:, ct * 128:(ct + 1) * 128],
                        ident[0:B, 0:B])
                    mtb = fin.tile([128, B], BF16, tag="mtb")
                    nc.scalar.copy(mtb[:], ptc[:])
                    nc.tensor.matmul(pfin[:], mtb[:], wm[ct][:],
                                     start=False, stop=(ct == NLT - 1),
                                     skip_group_check=True)
                ot = fin.tile([B, ESL], F32, tag="ot")
                nc.scalar.activation(ot[:], pfin[:], ACTF.Tanh)
                nc.sync.dma_start(out_sl[:], ot[:])
    nc.finalize()
    return nc


def _get_nc():
    global _nc_cache
    if _nc_cache is None:
        _nc_cache = _build()
    return _nc_cache


def _make_in_maps(inputs):
    import ml_dtypes
    query = np.asarray(inputs["query"], np.float32).reshape(B, D)
    qryT = np.ascontiguousarray(query.T)
    context = np.ascontiguousarray(np.asarray(inputs["context"], np.float32))
    delta_t = np.asarray(inputs["delta_t"], np.float32)
    W_in = np.asarray(inputs["W_in"], np.float32)
    W_out = np.asarray(inputs["W_out"], np.float32)
    ae = np.asarray(inputs["ae"], np.float32).reshape(B)
    ab = np.asarray(inputs["ab"], np.float32).reshape(B)
    # bt = exp(-ab*dt), transposed per batch to [128 partitions, NLT]
    bt = np.exp(-ab[:, None] * delta_t)                       # [B, L]
    btT = np.ascontiguousarray(
        bt.reshape(B, NLT, 128).transpose(0, 2, 1))           # [B, 128, NLT]
    in_maps = []
    for c in range(N_CORES):
        es = slice(c * ESL, (c + 1) * ESL)
        in_maps.append({
            "ctx": context[c * BLOC:(c + 1) * BLOC],
            "qryT": qryT,
            "w_inT": np.ascontiguousarray(W_in[es, :].T),
            "wmT": np.ascontiguousarray(
                W_out[es, 0:D].T).astype(ml_dtypes.bfloat16),
            "wqT": np.ascontiguousarray(
                W_out[es, D:2 * D].T).astype(ml_dtypes.bfloat16),
            "btT": np.ascontiguousarray(btT[c * BLOC:(c + 1) * BLOC]),
            "aeb": np.ascontiguousarray(ae[c * BLOC:(c + 1) * BLOC, None]),
        })
    return in_maps


def kernel(query, context, delta_t, W_in, W_out, ae, ab):
    from concourse.bass_utils import run_bass_kernel_spmd

    nc = _get_nc()
    in_maps = _make_in_maps(dict(query=query, context=context,
                                 delta_t=delta_t, W_in=W_in, W_out=W_out,
                                 ae=ae, ab=ab))
    res = run_bass_kernel_spmd(nc, in_maps, list(range(N_CORES))).results

    out = np.concatenate([res[c]["out_sl"] for c in range(N_CORES)], axis=1)
    attn = np.concatenate([res[c]["attn_out"] for c in range(N_CORES)], axis=0)
    return out.reshape(B, 1, D), attn.reshape(B, 1, L)


# revision 10
# speedup vs baseline: 4.4199x; 4.4199x over previous
"""Trainium2 Bass kernel for nn_AttentionHawkes (B=32, L=2048, D=2048, 8 cores).

Sharding: batch-parallel for context (4 batches per core), output-dim-parallel
for W_in/W_out (256 cols per core). Host pre-transposes weights/query and
precomputes bt=exp(-ab*dt).

Per batch on its core:
  pass A: scores[l] = x[l,:]@q via DVE scalar_tensor_tensor accumulate (f32),
          xb = bf16 copy of x on ACT; softmax via small PE matmuls.
  pass B (per tile, all bf16): rp = relu(c2*xb) on GpSimd tensor_scalar
          (c2 = ae*attn*bt, signed); first KZ d-chunks: z = attn*xb + rp
          fused on DVE then one PE matmul per chunk; remaining chunks:
          two PE matmuls per chunk (attn-weighted xb + ones-weighted rp)
          accumulating into the same PSUM row.
Final: out = tanh([mix|q] @ W_out^T) with the q-half accumulated into a
persistent PSUM bank at startup and the mix-half added in a short tail
after one AllGather.
"""
import sys, os
sys.path.insert(0, "/opt/trn_rl_repo")
import numpy as np

N_CORES = 8
B, L, D = 32, 2048, 2048
BLOC = B // N_CORES          # 4 batches per core
ESL = D // N_CORES           # 256 e-cols of W_in / W_out per core
NLT = L // 128               # 16 l-tiles per batch
NDC = D // 512               # 4 d-chunks of 512

_nc_cache = None


def _build():
    KZ = int(os.environ.get('KZ', '2'))   # d-chunks using the fused-z path
    import concourse.mybir as mybir
    import concourse.tile as tile
    from concourse import bacc
    from concourse.masks import make_identity

    F32 = mybir.dt.float32
    F32R = mybir.dt.float32r
    BF16 = mybir.dt.bfloat16
    ALU = mybir.AluOpType
    ACTF = mybir.ActivationFunctionType
    AX = mybir.AxisListType

    nc = bacc.Bacc()

    ctx = nc.dram_tensor("ctx", [BLOC, L, D], F32, kind="ExternalInput")
    qryT = nc.dram_tensor("qryT", [D, B], F32, kind="ExternalInput")
    w_inT = nc.dram_tensor("w_inT", [D, ESL], F32, kind="ExternalInput")
    wmT = nc.dram_tensor("wmT", [D, ESL], BF16, kind="ExternalInput")
    wqT = nc.dram_tensor("wqT", [D, ESL], BF16, kind="ExternalInput")
    btT_in = nc.dram_tensor("btT", [BLOC, 128, NLT], F32, kind="ExternalInput")
    aeb = nc.dram_tensor("aeb", [BLOC, 1], F32, kind="ExternalInput")

    out_sl = nc.dram_tensor("out_sl", [B, ESL], F32, kind="ExternalOutput")
    attn_out = nc.dram_tensor("attn_out", [BLOC, L], F32, kind="ExternalOutput")

    qg_in = nc.dram_tensor("qg_in", [B, ESL], F32)
    qloc = nc.dram_tensor("qloc", [N_CORES, BLOC, ESL], F32)
    qall = nc.dram_tensor("qall", [N_CORES, B, ESL], F32, addr_space="Shared")
    mix_in = nc.dram_tensor("mix_in", [BLOC, D], F32)
    mix_all = nc.dram_tensor("mix_all", [B, D], F32, addr_space="Shared")

    groups = [list(range(N_CORES))]

    with tile.TileContext(nc) as tc:
        with (
            tc.tile_pool(name="cpool", bufs=1) as cpool,
            tc.tile_pool(name="wout", bufs=1) as wout_pool,
            tc.tile_pool(name="pfin", bufs=1, space="PSUM") as pfin_pool,
            tc.tile_pool(name="fin", bufs=1) as fin,
        ):
            ident = cpool.tile([128, 128], F32)
            make_identity(nc, ident[:])
            ones_row = cpool.tile([1, 128], F32)
            nc.vector.memset(ones_row[:], 1.0)
            ones_col = cpool.tile([128, 1], F32)
            nc.vector.memset(ones_col[:], 1.0)
            ones_bf = cpool.tile([128, 2], BF16)
            nc.vector.memset(ones_bf[:], 1.0)

            # resident transposed W_out halves (bf16, 2MB)
            wm = []
            wq = []
            for ct in range(NLT):
                t1 = wout_pool.tile([128, ESL], BF16, tag="wm", name=f"wm{ct}")
                nc.sync.dma_start(t1[:], wmT[ct * 128:(ct + 1) * 128, :])
                wm.append(t1)
            for ct in range(NLT):
                t2 = wout_pool.tile([128, ESL], BF16, tag="wq", name=f"wq{ct}")
                nc.sync.dma_start(t2[:], wqT[ct * 128:(ct + 1) * 128, :])
                wq.append(t2)

            # persistent output-accumulator bank: q-half at startup,
            # mix-half in the tail
            pfin = pfin_pool.tile([B, ESL], F32)

            # ---------- startup: q = query @ W_in[eslice]^T, collectives ----
            with (
                tc.tile_pool(name="wst", bufs=1) as wst,
                tc.tile_pool(name="qst", bufs=1) as qst,
                tc.tile_pool(name="pst", bufs=2, space="PSUM") as pst,
            ):
                wi = []
                for t in range(NLT):
                    wt = wst.tile([128, ESL], F32, tag="wi", name=f"wi{t}")
                    nc.sync.dma_start(wt[:], w_inT[t * 128:(t + 1) * 128, :])
                    wi.append(wt)
                qT_sb = qst.tile([128, NLT * B], F32)
                for t in range(NLT):
                    nc.sync.dma_start(qT_sb[:, t * B:(t + 1) * B],
                                      qryT[t * 128:(t + 1) * 128, :])
                pq = pst.tile([B, ESL], F32, tag="pq")
                for t in range(NLT):
                    nc.tensor.matmul(pq[:], qT_sb[:, t * B:(t + 1) * B],
                                     wi[t][:], start=(t == 0),
                                     stop=(t == NLT - 1))
                q_sb = qst.tile([B, ESL], F32)
                nc.scalar.copy(q_sb[:], pq[:])
                nc.sync.dma_start(qg_in[:], q_sb[:])
                nc.gpsimd.collective_compute(
                    "AllToAll", ALU.bypass, replica_groups=groups,
                    ins=[qg_in.ap().opt()], outs=[qloc.ap().opt()])
                nc.gpsimd.collective_compute(
                    "AllGather", ALU.bypass, replica_groups=groups,
                    ins=[qg_in.ap().opt()], outs=[qall.ap().opt()])

                # qT (all batches, bf16) -> start the pfin chain with q-half
                for ct in range(NLT):
                    i, j = divmod(ct, ESL // 128)
                    qa = qst.tile([B, 128], F32, tag="qa")
                    nc.sync.dma_start(
                        qa[:], qall[i, :, j * 128:(j + 1) * 128])
                    ptq = pst.tile([128, B], F32, tag="ptq")
                    nc.tensor.transpose(ptq[:], qa[:], ident[0:B, 0:B])
                    qtb = qst.tile([128, B], BF16, tag="qtb")
                    nc.scalar.copy(qtb[:], ptq[:])
                    nc.tensor.matmul(pfin[:], qtb[:], wq[ct][:],
                                     start=(ct == 0), stop=False,
                                     skip_group_check=True)

            # ---------- main pools ----------
            with (
                tc.tile_pool(name="xp", bufs=NLT) as xp,
                tc.tile_pool(name="scr", bufs=2) as scr_pool,
                tc.tile_pool(name="rp", bufs=2) as rp_pool,
                tc.tile_pool(name="rn", bufs=2) as rn_pool,
                tc.tile_pool(name="qb", bufs=2) as qb_pool,
                tc.tile_pool(name="small", bufs=2) as small,
                tc.tile_pool(name="pm", bufs=1, space="PSUM") as pm_pool,
                tc.tile_pool(name="ptr", bufs=1, space="PSUM") as ptr_pool,
            ):
                for b in range(BLOC):
                    # q broadcast rows for the stt
                    qb = qb_pool.tile([128, D], F32, tag="qb")
                    for i in range(N_CORES):
                        nc.sync.dma_start(
                            qb[:, i * ESL:(i + 1) * ESL],
                            qloc[i:i + 1, b, :].broadcast_to([128, ESL]))

                    btb = small.tile([128, NLT], F32, tag="btb")
                    nc.sync.dma_start(btb[:], btT_in[b])
                    ae_col = small.tile([128, 1], F32, tag="ae_col")
                    nc.sync.dma_start(ae_col[:],
                                      aeb[b:b + 1, 0:1].broadcast_to([128, 1]))

                    # pass A: load x tiles + scores
                    scores = small.tile([128, NLT], F32, tag="scores")
                    xts = []
                    for t in range(NLT):
                        xt = xp.tile([128, D], F32, tag="xt")
                        nc.sync.dma_start(xt[:], ctx[b, t * 128:(t + 1) * 128, :])
                        scr = scr_pool.tile([128, D], BF16, tag="scr")
                        nc.vector.scalar_tensor_tensor(
                            out=scr[:], in0=xt[:], scalar=1.0, in1=qb[:],
                            op0=ALU.mult, op1=ALU.mult,
                            accum_out=scores[:, t:t + 1])
                        xts.append(xt)

                    # softmax over all 2048 scores (PE partition reduces)
                    m1 = small.tile([128, 1], F32, tag="m1")
                    nc.vector.reduce_max(m1[:], scores[:], axis=AX.X)
                    ptm = ptr_pool.tile([1, 128], F32, tag="ptr")
                    nc.tensor.transpose(ptm[:], m1[:], ident[:])
                    mg = small.tile([1, 1], F32, tag="mg")
                    nc.vector.reduce_max(mg[:], ptm[:], axis=AX.X)
                    nc.vector.tensor_scalar_mul(mg[:], mg[:], -1.0)
                    pnb = ptr_pool.tile([128, 1], F32, tag="ptr2")
                    nc.tensor.matmul(pnb[:], ones_row[:], mg[:],
                                     start=True, stop=True)
                    negm = small.tile([128, 1], F32, tag="negm")
                    nc.scalar.copy(negm[:], pnb[:])
                    E = small.tile([128, NLT], F32, tag="E")
                    s1 = small.tile([128, 1], F32, tag="s1")
                    nc.scalar.activation(E[:], scores[:], ACTF.Exp,
                                         bias=negm[:], accum_out=s1[:])
                    pz = ptr_pool.tile([1, 1], F32, tag="ptr")
                    nc.tensor.matmul(pz[:], s1[:, 0:1], ones_col[:, 0:1],
                                     start=True, stop=True)
                    rzg = small.tile([1, 1], F32, tag="rzg")
                    nc.vector.reciprocal(rzg[:], pz[:])
                    prz = ptr_pool.tile([128, 1], F32, tag="ptr2")
                    nc.tensor.matmul(prz[:], ones_row[:], rzg[:],
                                     start=True, stop=True)
                    rz = small.tile([128, 1], F32, tag="rz")
                    nc.scalar.copy(rz[:], prz[:])
                    attn = small.tile([128, NLT], F32, tag="attn")
                    nc.vector.tensor_scalar(out=attn[:], in0=E[:],
                                            scalar1=rz[:], scalar2=None,
                                            op0=ALU.mult)

                    # coefficients: c2 = ae*attn*bt (signed),
                    # CP = attn + max(c2,0), CN = max(-c2,0) - attn (f32r)
                    c2 = small.tile([128, NLT], F32, tag="c2")
                    nc.vector.tensor_tensor(out=c2[:], in0=attn[:],
                                            in1=btb[:], op=ALU.mult)
                    nc.vector.tensor_scalar(out=c2[:], in0=c2[:],
                                            scalar1=ae_col[:], scalar2=None,
                                            op0=ALU.mult)
                    cp = small.tile([128, NLT], F32, tag="cp")
                    nc.vector.tensor_scalar(out=cp[:], in0=c2[:], scalar1=0.0,
                                            scalar2=None, op0=ALU.max)
                    cp_r = small.tile([128, NLT], F32R, tag="cp_r")
                    nc.vector.tensor_tensor(out=cp_r[:], in0=cp[:],
                                            in1=attn[:], op=ALU.add)
                    cn = small.tile([128, NLT], F32, tag="cn")
                    nc.vector.tensor_scalar(out=cn[:], in0=c2[:], scalar1=-1.0,
                                            scalar2=0.0, op0=ALU.mult,
                                            op1=ALU.max)
                    cn_r = small.tile([128, NLT], F32R, tag="cn_r")
                    nc.vector.tensor_tensor(out=cn_r[:], in0=cn[:],
                                            in1=attn[:], op=ALU.subtract)

                    # attn output (transpose to l-major)
                    pat = ptr_pool.tile([NLT, 128], F32, tag="ptr")
                    nc.tensor.transpose(pat[:], attn[:], ident[:])
                    at_sb = small.tile([NLT, 128], F32, tag="at_sb")
                    nc.scalar.copy(at_sb[:], pat[:])
                    nc.sync.dma_start(
                        attn_out[b].rearrange("(t p) -> t p", p=128), at_sb[:])

                    # pass B: accumulate CP*relu(x) + CN*relu(-x) over l
                    pms = [pm_pool.tile([2, 512], F32, tag=f"pm{dc}",
                                        name=f"pm{dc}")
                           for dc in range(NDC)]
                    for t in range(NLT):
                        rpl = rp_pool.tile([128, D], F32R, tag="rpl")
                        nc.scalar.activation(rpl[:], xts[t][:], ACTF.Relu)
                        rnl = rn_pool.tile([128, D], F32R, tag="rnl")
                        nc.vector.tensor_scalar(out=rnl[:], in0=xts[t][:],
                                                scalar1=-1.0, scalar2=0.0,
                                                op0=ALU.mult, op1=ALU.max)
                        for dc in range(NDC):
                            nc.tensor.matmul(
                                pms[dc][:],
                                cp_r[:, t:t + 1].broadcast_to([128, 2]),
                                rpl[:, dc * 512:(dc + 1) * 512],
                                start=(t == 0), stop=False)
                        for dc in range(NDC):
                            nc.tensor.matmul(
                                pms[dc][:],
                                cn_r[:, t:t + 1].broadcast_to([128, 2]),
                                rnl[:, dc * 512:(dc + 1) * 512],
                                start=False, stop=(t == NLT - 1))

                    # mix row -> DRAM
                    ms = fin.tile([1, D], F32, tag="ms")
                    for dc in range(NDC):
                        nc.scalar.copy(ms[:, dc * 512:(dc + 1) * 512],
                                       pms[dc][0:1, :])
                    nc.sync.dma_start(mix_in[b:b + 1, :], ms[0:1, :])

                # ---------- tail: gather mix, finish pfin, tanh ----------
                nc.gpsimd.collective_compute(
                    "AllGather", ALU.bypass, replica_groups=groups,
                    ins=[mix_in.ap().opt()], outs=[mix_all.ap().opt()])
                comb_sb = fin.tile([B, D], F32, tag="comb")
                nc.sync.dma_start(comb_sb[:], mix_all[:])
                for ct in range(NLT):
                    ptc = ptr_pool.tile([128, B], F32, tag="ptr3")
                    nc.tensor.transpose(
                        ptc[:], comb_sb[:, ct * 128:(ct + 1) * 128],
                        ident[0:B, 0:B])
                    mtb = fin.tile([128, B], BF16, tag="mtb")
                    nc.scalar.copy(mtb[:], ptc[:])
                    nc.tensor.matmul(pfin[:], mtb[:], wm[ct][:],
                                     start=False, stop=(ct == NLT - 1),
                                     skip_group_check=True)
                ot = fin.tile([B, ESL], F32, tag="ot")
                nc.scalar.activation(ot[:], pfin[:], ACTF.Tanh)
                nc.sync.dma_start(out_sl[:], ot[:])
    nc.finalize()
    return nc


def _get_nc():
    global _nc_cache
    if _nc_cache is None:
        _nc_cache = _build()
    return _nc_cache


def _make_in_maps(inputs):
    import ml_dtypes
    query = np.asarray(inputs["query"], np.float32).reshape(B, D)
    qryT = np.ascontiguousarray(query.T)
    context = np.ascontiguousarray(np.asarray(inputs["context"], np.float32))
    delta_t = np.asarray(inputs["delta_t"], np.float32)
    W_in = np.asarray(inputs["W_in"], np.float32)
    W_out = np.asarray(inputs["W_out"], np.float32)
    ae = np.asarray(inputs["ae"], np.float32).reshape(B)
    ab = np.asarray(inputs["ab"], np.float32).reshape(B)
    # bt = exp(-ab*dt), transposed per batch to [128 partitions, NLT]
    bt = np.exp(-ab[:, None] * delta_t)                       # [B, L]
    btT = np.ascontiguousarray(
        bt.reshape(B, NLT, 128).transpose(0, 2, 1))           # [B, 128, NLT]
    in_maps = []
    for c in range(N_CORES):
        es = slice(c * ESL, (c + 1) * ESL)
        in_maps.append({
            "ctx": context[c * BLOC:(c + 1) * BLOC],
            "qryT": qryT,
            "w_inT": np.ascontiguousarray(W_in[es, :].T),
            "wmT": np.ascontiguousarray(
                W_out[es, 0:D].T).astype(ml_dtypes.bfloat16),
            "wqT": np.ascontiguousarray(
                W_out[es, D:2 * D].T).astype(ml_dtypes.bfloat16),
            "btT": np.ascontiguousarray(btT[c * BLOC:(c + 1) * BLOC]),
            "aeb": np.ascontiguousarray(ae[c * BLOC:(c + 1) * BLOC, None]),
        })
    return in_maps


def kernel(query, context, delta_t, W_in, W_out, ae, ab):
    from concourse.bass_utils import run_bass_kernel_spmd

    nc = _get_nc()
    in_maps = _make_in_maps(dict(query=query, context=context,
                                 delta_t=delta_t, W_in=W_in, W_out=W_out,
                                 ae=ae, ab=ab))
    res = run_bass_kernel_spmd(nc, in_maps, list(range(N_CORES))).results

    out = np.concatenate([res[c]["out_sl"] for c in range(N_CORES)], axis=1)
    attn = np.concatenate([res[c]["attn_out"] for c in range(N_CORES)], axis=0)
    return out.reshape(B, 1, D), attn.reshape(B, 1, L)


# revision 11
# speedup vs baseline: 4.5551x; 1.0306x over previous
"""Trainium2 Bass kernel for nn_AttentionHawkes (B=32, L=2048, D=2048, 8 cores).

Sharding: batch-parallel for context (4 batches per core), output-dim-parallel
for W_in/W_out (256 cols per core). Host pre-transposes weights/query and
precomputes bt=exp(-ab*dt).

Per batch on its core:
  pass A: scores[l] = x[l,:]@q via DVE scalar_tensor_tensor accumulate (f32),
          xb = bf16 copy of x on ACT; softmax via small PE matmuls.
  pass B (per tile, all bf16): rp = relu(c2*xb) on GpSimd tensor_scalar
          (c2 = ae*attn*bt, signed); first KZ d-chunks: z = attn*xb + rp
          fused on DVE then one PE matmul per chunk; remaining chunks:
          two PE matmuls per chunk (attn-weighted xb + ones-weighted rp)
          accumulating into the same PSUM row.
Final: out = tanh([mix|q] @ W_out^T) with the q-half accumulated into a
persistent PSUM bank at startup and the mix-half added in a short tail
after one AllGather.
"""
import sys, os
sys.path.insert(0, "/opt/trn_rl_repo")
import numpy as np

N_CORES = 8
B, L, D = 32, 2048, 2048
BLOC = B // N_CORES          # 4 batches per core
ESL = D // N_CORES           # 256 e-cols of W_in / W_out per core
NLT = L // 128               # 16 l-tiles per batch
NDC = D // 512               # 4 d-chunks of 512

_nc_cache = None


def _build():
    KZ = int(os.environ.get('KZ', '2'))   # d-chunks using the fused-z path
    import concourse.mybir as mybir
    import concourse.tile as tile
    from concourse import bacc
    from concourse.masks import make_identity

    F32 = mybir.dt.float32
    F32R = mybir.dt.float32r
    BF16 = mybir.dt.bfloat16
    ALU = mybir.AluOpType
    ACTF = mybir.ActivationFunctionType
    AX = mybir.AxisListType

    nc = bacc.Bacc()

    ctx = nc.dram_tensor("ctx", [BLOC, L, D], F32, kind="ExternalInput")
    qryT = nc.dram_tensor("qryT", [D, B], F32, kind="ExternalInput")
    w_inT = nc.dram_tensor("w_inT", [D, ESL], F32, kind="ExternalInput")
    wmT = nc.dram_tensor("wmT", [D, ESL], BF16, kind="ExternalInput")
    wqT = nc.dram_tensor("wqT", [D, ESL], BF16, kind="ExternalInput")
    btT_in = nc.dram_tensor("btT", [BLOC, 128, NLT], F32, kind="ExternalInput")
    aeb = nc.dram_tensor("aeb", [BLOC, 1], F32, kind="ExternalInput")

    out_sl = nc.dram_tensor("out_sl", [B, ESL], F32, kind="ExternalOutput")
    attn_out = nc.dram_tensor("attn_out", [BLOC, L], F32, kind="ExternalOutput")

    qg_in = nc.dram_tensor("qg_in", [B, ESL], F32)
    qloc = nc.dram_tensor("qloc", [N_CORES, BLOC, ESL], F32)
    qall = nc.dram_tensor("qall", [N_CORES, B, ESL], F32, addr_space="Shared")
    mix_in = nc.dram_tensor("mix_in", [BLOC, D], F32)
    mix_all = nc.dram_tensor("mix_all", [B, D], F32, addr_space="Shared")

    groups = [list(range(N_CORES))]

    with tile.TileContext(nc) as tc:
        with (
            tc.tile_pool(name="cpool", bufs=1) as cpool,
            tc.tile_pool(name="wout", bufs=1) as wout_pool,
            tc.tile_pool(name="pfin", bufs=1, space="PSUM") as pfin_pool,
            tc.tile_pool(name="fin", bufs=1) as fin,
        ):
            ident = cpool.tile([128, 128], F32)
            make_identity(nc, ident[:])
            ones_row = cpool.tile([1, 128], F32)
            nc.vector.memset(ones_row[:], 1.0)
            ones_col = cpool.tile([128, 1], F32)
            nc.vector.memset(ones_col[:], 1.0)
            ones_bf = cpool.tile([128, 2], BF16)
            nc.vector.memset(ones_bf[:], 1.0)

            # resident transposed W_out halves (bf16, 2MB)
            wm = []
            wq = []
            for ct in range(NLT):
                t1 = wout_pool.tile([128, ESL], BF16, tag="wm", name=f"wm{ct}")
                nc.sync.dma_start(t1[:], wmT[ct * 128:(ct + 1) * 128, :])
                wm.append(t1)
            for ct in range(NLT):
                t2 = wout_pool.tile([128, ESL], BF16, tag="wq", name=f"wq{ct}")
                nc.sync.dma_start(t2[:], wqT[ct * 128:(ct + 1) * 128, :])
                wq.append(t2)

            # persistent output-accumulator bank: q-half at startup,
            # mix-half in the tail
            pfin = pfin_pool.tile([B, ESL], F32)

            # ---------- startup: q = query @ W_in[eslice]^T, collectives ----
            with (
                tc.tile_pool(name="wst", bufs=1) as wst,
                tc.tile_pool(name="qst", bufs=1) as qst,
                tc.tile_pool(name="pst", bufs=2, space="PSUM") as pst,
            ):
                wi = []
                for t in range(NLT):
                    wt = wst.tile([128, ESL], F32, tag="wi", name=f"wi{t}")
                    nc.sync.dma_start(wt[:], w_inT[t * 128:(t + 1) * 128, :])
                    wi.append(wt)
                qT_sb = qst.tile([128, NLT * B], F32)
                for t in range(NLT):
                    nc.sync.dma_start(qT_sb[:, t * B:(t + 1) * B],
                                      qryT[t * 128:(t + 1) * 128, :])
                pq = pst.tile([B, ESL], F32, tag="pq")
                for t in range(NLT):
                    nc.tensor.matmul(pq[:], qT_sb[:, t * B:(t + 1) * B],
                                     wi[t][:], start=(t == 0),
                                     stop=(t == NLT - 1))
                q_sb = qst.tile([B, ESL], F32)
                nc.scalar.copy(q_sb[:], pq[:])
                nc.sync.dma_start(qg_in[:], q_sb[:])
                nc.gpsimd.collective_compute(
                    "AllToAll", ALU.bypass, replica_groups=groups,
                    ins=[qg_in.ap().opt()], outs=[qloc.ap().opt()])
                nc.gpsimd.collective_compute(
                    "AllGather", ALU.bypass, replica_groups=groups,
                    ins=[qg_in.ap().opt()], outs=[qall.ap().opt()])

                # qT (all batches, bf16) -> start the pfin chain with q-half
                for ct in range(NLT):
                    i, j = divmod(ct, ESL // 128)
                    qa = qst.tile([B, 128], F32, tag="qa")
                    nc.sync.dma_start(
                        qa[:], qall[i, :, j * 128:(j + 1) * 128])
                    ptq = pst.tile([128, B], F32, tag="ptq")
                    nc.tensor.transpose(ptq[:], qa[:], ident[0:B, 0:B])
                    qtb = qst.tile([128, B], BF16, tag="qtb")
                    nc.scalar.copy(qtb[:], ptq[:])
                    nc.tensor.matmul(pfin[:], qtb[:], wq[ct][:],
                                     start=(ct == 0), stop=False,
                                     skip_group_check=True)

            # ---------- main pools ----------
            # Software-pipelined: iteration `it` runs pass A of batch `it`
            # (DMA + scores + bf16 relu streams) interleaved tile-by-tile
            # with pass B matmuls of batch `it-1`.
            with (
                tc.tile_pool(name="xp", bufs=4) as xp,
                tc.tile_pool(name="scr", bufs=1) as scr_pool,
                tc.tile_pool(name="rp", bufs=NLT) as rp_pool,
                tc.tile_pool(name="rn", bufs=NLT) as rn_pool,
                tc.tile_pool(name="qb", bufs=2) as qb_pool,
                tc.tile_pool(name="small", bufs=2) as small,
                tc.tile_pool(name="pm", bufs=1, space="PSUM") as pm_pool,
                tc.tile_pool(name="ptr", bufs=1, space="PSUM") as ptr_pool,
            ):
                prev = None   # (rpls, rnls, cp_r, cn_r) of batch it-1
                for it in range(BLOC + 1):
                    b = it if it < BLOC else None
                    if b is not None:
                        qb = qb_pool.tile([128, D], F32, tag="qb")
                        for i in range(N_CORES):
                            nc.sync.dma_start(
                                qb[:, i * ESL:(i + 1) * ESL],
                                qloc[i:i + 1, b, :].broadcast_to([128, ESL]))
                        btb = small.tile([128, NLT], F32, tag="btb")
                        nc.sync.dma_start(btb[:], btT_in[b])
                        ae_col = small.tile([128, 1], F32, tag="ae_col")
                        nc.sync.dma_start(
                            ae_col[:], aeb[b:b + 1, 0:1].broadcast_to([128, 1]))
                        scores = small.tile([128, NLT], F32, tag="scores")

                    if prev is not None:
                        pms = [pm_pool.tile([2, 512], F32, tag=f"pm{dc}",
                                            name=f"pm{dc}")
                               for dc in range(NDC)]

                    rpls = []
                    rnls = []
                    for t in range(NLT):
                        if b is not None:
                            xt = xp.tile([128, D], F32, tag="xt")
                            nc.sync.dma_start(
                                xt[:], ctx[b, t * 128:(t + 1) * 128, :])
                            scr = scr_pool.tile([128, D], BF16, tag="scr")
                            nc.vector.scalar_tensor_tensor(
                                out=scr[:], in0=xt[:], scalar=1.0, in1=qb[:],
                                op0=ALU.mult, op1=ALU.mult,
                                accum_out=scores[:, t:t + 1])
                            rpl = rp_pool.tile([128, D], BF16, tag="rpl")
                            nc.scalar.activation(rpl[:], xt[:], ACTF.Relu)
                            rnl = rn_pool.tile([128, D], BF16, tag="rnl")
                            nc.vector.tensor_scalar(
                                out=rnl[:], in0=xt[:], scalar1=-1.0,
                                scalar2=0.0, op0=ALU.mult, op1=ALU.max)
                            rpls.append(rpl)
                            rnls.append(rnl)
                        if prev is not None:
                            prpl, prnl, pcp, pcn = prev
                            for dc in range(NDC):
                                nc.tensor.matmul(
                                    pms[dc][:],
                                    pcp[:, t:t + 1].broadcast_to([128, 2]),
                                    prpl[t][:, dc * 512:(dc + 1) * 512],
                                    start=(t == 0), stop=False)
                            for dc in range(NDC):
                                nc.tensor.matmul(
                                    pms[dc][:],
                                    pcn[:, t:t + 1].broadcast_to([128, 2]),
                                    prnl[t][:, dc * 512:(dc + 1) * 512],
                                    start=False, stop=(t == NLT - 1))

                    if prev is not None:
                        # mix row of batch it-1 -> DRAM
                        ms = fin.tile([1, D], F32, tag="ms")
                        for dc in range(NDC):
                            nc.scalar.copy(ms[:, dc * 512:(dc + 1) * 512],
                                           pms[dc][0:1, :])
                        nc.sync.dma_start(mix_in[it - 1:it, :], ms[0:1, :])

                    if b is None:
                        break

                    # softmax over all 2048 scores (PE partition reduces)
                    m1 = small.tile([128, 1], F32, tag="m1")
                    nc.vector.reduce_max(m1[:], scores[:], axis=AX.X)
                    ptm = ptr_pool.tile([1, 128], F32, tag="ptr")
                    nc.tensor.transpose(ptm[:], m1[:], ident[:])
                    mg = small.tile([1, 1], F32, tag="mg")
                    nc.vector.reduce_max(mg[:], ptm[:], axis=AX.X)
                    nc.vector.tensor_scalar_mul(mg[:], mg[:], -1.0)
                    pnb = ptr_pool.tile([128, 1], F32, tag="ptr2")
                    nc.tensor.matmul(pnb[:], ones_row[:], mg[:],
                                     start=True, stop=True)
                    negm = small.tile([128, 1], F32, tag="negm")
                    nc.scalar.copy(negm[:], pnb[:])
                    E = small.tile([128, NLT], F32, tag="E")
                    s1 = small.tile([128, 1], F32, tag="s1")
                    nc.scalar.activation(E[:], scores[:], ACTF.Exp,
                                         bias=negm[:], accum_out=s1[:])
                    pz = ptr_pool.tile([1, 1], F32, tag="ptr")
                    nc.tensor.matmul(pz[:], s1[:, 0:1], ones_col[:, 0:1],
                                     start=True, stop=True)
                    rzg = small.tile([1, 1], F32, tag="rzg")
                    nc.vector.reciprocal(rzg[:], pz[:])
                    prz = ptr_pool.tile([128, 1], F32, tag="ptr2")
                    nc.tensor.matmul(prz[:], ones_row[:], rzg[:],
                                     start=True, stop=True)
                    rz = small.tile([128, 1], F32, tag="rz")
                    nc.scalar.copy(rz[:], prz[:])
                    attn = small.tile([128, NLT], F32, tag="attn")
                    nc.vector.tensor_scalar(out=attn[:], in0=E[:],
                                            scalar1=rz[:], scalar2=None,
                                            op0=ALU.mult)

                    # coefficients: c2 = ae*attn*bt (signed),
                    # CP = attn + max(c2,0), CN = max(-c2,0) - attn (bf16)
                    c2 = small.tile([128, NLT], F32, tag="c2")
                    nc.vector.tensor_tensor(out=c2[:], in0=attn[:],
                                            in1=btb[:], op=ALU.mult)
                    nc.vector.tensor_scalar(out=c2[:], in0=c2[:],
                                            scalar1=ae_col[:], scalar2=None,
                                            op0=ALU.mult)
                    cp = small.tile([128, NLT], F32, tag="cp")
                    nc.vector.tensor_scalar(out=cp[:], in0=c2[:], scalar1=0.0,
                                            scalar2=None, op0=ALU.max)
                    cp_r = small.tile([128, NLT], BF16, tag="cp_r")
                    nc.vector.tensor_tensor(out=cp_r[:], in0=cp[:],
                                            in1=attn[:], op=ALU.add)
                    cn = small.tile([128, NLT], F32, tag="cn")
                    nc.vector.tensor_scalar(out=cn[:], in0=c2[:], scalar1=-1.0,
                                            scalar2=0.0, op0=ALU.mult,
                                            op1=ALU.max)
                    cn_r = small.tile([128, NLT], BF16, tag="cn_r")
                    nc.vector.tensor_tensor(out=cn_r[:], in0=cn[:],
                                            in1=attn[:], op=ALU.subtract)

                    # attn output (transpose to l-major)
                    pat = ptr_pool.tile([NLT, 128], F32, tag="ptr")
                    nc.tensor.transpose(pat[:], attn[:], ident[:])
                    at_sb = small.tile([NLT, 128], F32, tag="at_sb")
                    nc.scalar.copy(at_sb[:], pat[:])
                    nc.sync.dma_start(
                        attn_out[b].rearrange("(t p) -> t p", p=128), at_sb[:])

                    prev = (rpls, rnls, cp_r, cn_r)

                # ---------- tail: gather mix, finish pfin, tanh ----------
                nc.gpsimd.collective_compute(
                    "AllGather", ALU.bypass, replica_groups=groups,
                    ins=[mix_in.ap().opt()], outs=[mix_all.ap().opt()])
                comb_sb = fin.tile([B, D], F32, tag="comb")
                nc.sync.dma_start(comb_sb[:], mix_all[:])
                for ct in range(NLT):
                    ptc = ptr_pool.tile([128, B], F32, tag="ptr3")
                    nc.tensor.transpose(
                        ptc[:], comb_sb[:, ct * 128:(ct + 1) * 128],
                        ident[0:B, 0:B])
                    mtb = fin.tile([128, B], BF16, tag="mtb")
                    nc.scalar.copy(mtb[:], ptc[:])
                    nc.tensor.matmul(pfin[:], mtb[:], wm[ct][:],
                                     start=False, stop=(ct == NLT - 1),
                                     skip_group_check=True)
                ot = fin.tile([B, ESL], F32, tag="ot")
                nc.scalar.activation(ot[:], pfin[:], ACTF.Tanh)
                nc.sync.dma_start(out_sl[:], ot[:])
    nc.finalize()
    return nc


def _get_nc():
    global _nc_cache
    if _nc_cache is None:
        _nc_cache = _build()
    return _nc_cache


def _make_in_maps(inputs):
    import ml_dtypes
    query = np.asarray(inputs["query"], np.float32).reshape(B, D)
    qryT = np.ascontiguousarray(query.T)
    context = np.ascontiguousarray(np.asarray(inputs["context"], np.float32))
    delta_t = np.asarray(inputs["delta_t"], np.float32)
    W_in = np.asarray(inputs["W_in"], np.float32)
    W_out = np.asarray(inputs["W_out"], np.float32)
    ae = np.asarray(inputs["ae"], np.float32).reshape(B)
    ab = np.asarray(inputs["ab"], np.float32).reshape(B)
    # bt = exp(-ab*dt), transposed per batch to [128 partitions, NLT]
    bt = np.exp(-ab[:, None] * delta_t)                       # [B, L]
    btT = np.ascontiguousarray(
        bt.reshape(B, NLT, 128).transpose(0, 2, 1))           # [B, 128, NLT]
    in_maps = []
    for c in range(N_CORES):
        es = slice(c * ESL, (c + 1) * ESL)
        in_maps.append({
            "ctx": context[c * BLOC:(c + 1) * BLOC],
            "qryT": qryT,
            "w_inT": np.ascontiguousarray(W_in[es, :].T),
            "wmT": np.ascontiguousarray(
                W_out[es, 0:D].T).astype(ml_dtypes.bfloat16),
            "wqT": np.ascontiguousarray(
                W_out[es, D:2 * D].T).astype(ml_dtypes.bfloat16),
            "btT": np.ascontiguousarray(btT[c * BLOC:(c + 1) * BLOC]),
            "aeb": np.ascontiguousarray(ae[c * BLOC:(c + 1) * BLOC, None]),
        })
    return in_maps


def kernel(query, context, delta_t, W_in, W_out, ae, ab):
    from concourse.bass_utils import run_bass_kernel_spmd

    nc = _get_nc()
    in_maps = _make_in_maps(dict(query=query, context=context,
                                 delta_t=delta_t, W_in=W_in, W_out=W_out,
                                 ae=ae, ab=ab))
    res = run_bass_kernel_spmd(nc, in_maps, list(range(N_CORES))).results

    out = np.concatenate([res[c]["out_sl"] for c in range(N_CORES)], axis=1)
    attn = np.concatenate([res[c]["attn_out"] for c in range(N_CORES)], axis=0)
    return out.reshape(B, 1, D), attn.reshape(B, 1, L)
